# revision 39
# baseline (speedup 1.0000x reference)
"""Single-head causal attention (B=4, T=2048, C=2048, H=128) on 8 TRN2 cores.

Sharding: 2 cores per batch. T is split into 16 query tiles of 128 rows.
Core (2b + par) handles batch b and query tiles t in {par, par+2, ..., par+14}.
Query tile class i (i = 1..8) is processed with a padded causal key window of
2i key tiles, so every core executes an identical program; per-core inputs
carry the asymmetry.

Key-order permutation trick: the host reorders the T dimension of the per-core
x.T buffer as [own_1, sib_1, own_2, sib_2, ...] (own_i = the core's class-i
query tile, sib_i = the sibling core's). Attention sums are order-invariant
over keys, and the class-i key window is exactly the first 2i positions of
this order, so the program is position-based and identical across cores:
  - Q columns are the even positions (fixed offsets for every core),
  - the window's second-to-last position (even) is always the diagonal tile
    (constant triangular mask, built on device),
  - the last position (odd, the sibling tile) is all-allowed or all-masked
    depending only on core parity (a per-core [128,1] scalar input).

Per core on-device pipeline (all matmuls bf16, fp32 PSUM accumulation):
  phase 1 (DMA-paced): x.T arrives in 8 chunks, the first split in half so the
    first matmul waits on ~1MB. The weight stream (sync ring) and x stream
    (scalar ring) share the bandwidth-saturated DMA queues evenly, so each
    ring's order is arranged to deliver just in time (wq/odd trail the x
    stream). K^T, V^T accumulate per chunk.
  phase 2: Q^T (even positions, strided rhs AP); V tiles [k, h] via XBAR DMA
    transpose from the V^T staging (SBUF->SBUF, zero PE time)
  phase 3: attention, kt-outer over two class halves (cols [0,512), [512,1024)):
    S^T(kt) = K_kt.T @ Q^T[:, c0:]     (one N<=512 matmul)
    diagonal-class masks are ADDED (-1e4) into the S PSUM via an
    identity-lhsT matmul (negtri for even kt, the parity-scalar broadcast for
    odd kt), so exp -> O needs no vector hop
    A = exp(scale * S^T) on ACT (softmax max-shift skipped: |s| < ~6)
    O^T[half] += V_kt(as lhsT) @ A
    den: per kt PAIR, bf16 pair-sum of the two A tiles on VECTOR, then a
    [128,128]-ones matmul per pair (M=128 runs ~2x faster than an M=1 row
    reduce; row 0 of the result is the column sum), emitted one pair behind
    so a lagging vector add never stalls the in-order PE stream.
  outputs: ot [128, 1024] bf16 (host divides in fp32), den [1, 1024] f32 via
  the scalar engine (parallel with vector's ot cast); host computes
  (ot/den).T and scatters rows back.

PSUM is managed as one pool with 8 explicitly reused bank tags: phase 1
K 0-3 / V^T 4-7; Q 0-1; attention S rotates 3-6 (half1: 3-6,7,2), OT half0
on 7 / half1 on 0, den half0 on 2 / half1 on 1.

Cross-core notes: collective_compute and remote_dma both fault in this
sandbox's PJRT path (no communicator; NRT_EXEC_UNIT_UNRECOVERABLE), so the
K/V projection duplication across the core pair cannot be exchanged away.
"""

import numpy as np
import ml_dtypes

B, T, C, H = 4, 2048, 2048, 128
P = 128                 # tile edge
NCT = C // P            # 16 contraction chunks
NKT = T // P            # 16 key tiles / positions
NQT = 8                 # query tile classes per core
NQ = NQT * P            # 1024 query rows per core
N_CORES = 8
SCALE = float(H) ** -0.5
BF16 = ml_dtypes.bfloat16

_cache = {}


def _build():
    import concourse.bass as bass
    import concourse.mybir as mybir
    import concourse.tile as tile
    from concourse import bacc
    from concourse.masks import make_identity, make_lower_triangular

    dt = mybir.dt
    nc = bacc.Bacc(
        "TRN2",
        target_bir_lowering=False,
        debug=False,
        enable_asserts=False,
        num_devices=N_CORES,
    )

    xkvT = nc.dram_tensor("xkvT", [C, T], dt.bfloat16, kind="ExternalInput").ap()
    wq_d = nc.dram_tensor("wq", [P, NCT, H], dt.bfloat16, kind="ExternalInput").ap()
    wk_d = nc.dram_tensor("wk", [P, NCT, H], dt.bfloat16, kind="ExternalInput").ap()
    wv_d = nc.dram_tensor("wv", [P, NCT, H], dt.bfloat16, kind="ExternalInput").ap()
    # parity mask value: 0.0 if the sibling (odd-position) key tile is allowed
    # (par=1 cores), -10000.0 if masked (par=0 cores) -- added to scores
    # pre-exp via a PSUM-accumulating matmul
    oddn_d = nc.dram_tensor("oddn", [P, 1], dt.float32, kind="ExternalInput").ap()
    ot_d = nc.dram_tensor("ot", [H, NQ], dt.bfloat16, kind="ExternalOutput").ap()
    den_d = nc.dram_tensor("den", [1, NQ], dt.float32, kind="ExternalOutput").ap()

    XJ = 2          # c-tiles per x chunk
    NG = NCT // XJ  # 8 pipelined load/compute chunks

    with tile.TileContext(nc) as tc:
        with (
            tc.tile_pool(name="persist", bufs=1) as persist,
            tc.tile_pool(name="ephem", bufs=8) as ephem,
            tc.tile_pool(name="pair", bufs=8) as pairp,
            tc.tile_pool(name="outp", bufs=2) as outp,
            tc.tile_pool(name="psum", bufs=1, space="PSUM") as psum,
        ):
            def bank(b, shape=(P, 512), dtype=dt.float32, name="pb"):
                return psum.tile(list(shape), dtype, tag=f"bank{b}", name=f"{name}{b}")

            wq_sb = persist.tile([P, NCT, H], dt.bfloat16)
            wk_sb = persist.tile([P, NCT, H], dt.bfloat16)
            wv0_sb = persist.tile([P, 1, H], dt.bfloat16)
            wvr_sb = persist.tile([P, NCT - 1, H], dt.bfloat16)
            oddn_sb = persist.tile([P, 1], dt.float32)
            xg_sb = [
                persist.tile([P, XJ, T], dt.bfloat16, name=f"xg{g}")
                for g in range(NG)
            ]
            k_sb = persist.tile([P, T], dt.bfloat16)       # K^T [h, T]
            vt_sb = persist.tile([P, T], dt.bfloat16)      # V^T [h, T]
            v_sb = persist.tile([P, NKT, H], dt.bfloat16)  # V tiles [k, h]
            q_sb = persist.tile([P, NQ], dt.bfloat16)      # Q^T [h, NQ]
            ident = persist.tile([P, P], dt.bfloat16)
            negtri = persist.tile([P, P], dt.bfloat16)     # -1e4 where k > q
            oddneg = persist.tile([P, P], dt.bfloat16)     # oddn broadcast
            ones_sb = persist.tile([P, P], dt.bfloat16)    # den reducer lhsT

            def xdma(g):
                nc.scalar.dma_start(
                    out=xg_sb[g][:],
                    in_=xkvT[XJ * P * g:XJ * P * (g + 1), :].rearrange(
                        "(j p) t -> p j t", p=P
                    ),
                )

            def xdma_half(jj):
                nc.scalar.dma_start(
                    out=xg_sb[0][:, jj:jj + 1, :],
                    in_=xkvT[P * jj:P * (jj + 1), :]
                    .rearrange("(j p) t -> p j t", p=P),
                )

            # The 16 DMA queues are bandwidth-saturated through phase 1 and
            # the two HWDGE rings (sync: weights, scalar: x) share them about
            # evenly while both have work. Startup critical path: wk + the
            # first half of chunk 0 (~1MB) gates the first matmul; the wv
            # sliver covers chunk 0's V matmuls; wq/odd trail the x stream
            # (needed ~30us later, and by then the sync ring is drained so x
            # runs at full rate).
            nc.sync.dma_start(out=wk_sb[:], in_=wk_d[:])
            xdma_half(0)
            nc.sync.dma_start(out=wv0_sb[:], in_=wv_d[:, 0:1, :])
            xdma_half(1)
            nc.sync.dma_start(out=wvr_sb[:], in_=wv_d[:, 1:NCT, :])
            for g in range(1, NG):
                xdma(g)
            nc.scalar.dma_start(out=wq_sb[:], in_=wq_d[:])
            nc.scalar.dma_start(out=oddn_sb[:], in_=oddn_d[:])
            make_identity(nc, ident[:])
            make_lower_triangular(nc, negtri[:], val=-10000.0, diag=False)
            nc.vector.memset(oddneg[:], 1.0)
            nc.vector.tensor_scalar_mul(oddneg[:], oddneg[:], oddn_sb[:])
            nc.vector.memset(ones_sb[:], 1.0)
            # preload the ACT exp table off the attention critical path
            warm_sb = persist.tile([P, 1], dt.float32)
            nc.scalar.activation(
                warm_sb[:], ones_sb[:, 0:1], mybir.ActivationFunctionType.Exp
            )

            # ---- phase 1: pipelined x load + K^T / V^T accumulation ----
            # banks 0-3: K accum; banks 4-7: V^T accum
            ps_k = [bank(n, name="psk") for n in range(4)]
            ps_vt = [bank(4 + n, name="psvt") for n in range(4)]
            for g in range(NG):
                for jj in range(XJ):
                    j = XJ * g + jj
                    st, sp = j == 0, j == NCT - 1
                    wvj = wv0_sb[:, 0, :] if j == 0 else wvr_sb[:, j - 1, :]
                    for n in range(4):
                        nc.tensor.matmul(
                            ps_k[n][:],
                            lhsT=wk_sb[:, j, :],
                            rhs=xg_sb[g][:, jj, 512 * n:512 * (n + 1)],
                            start=st, stop=sp,
                        )
                    for n in range(4):
                        nc.tensor.matmul(
                            ps_vt[n][:],
                            lhsT=wvj,
                            rhs=xg_sb[g][:, jj, 512 * n:512 * (n + 1)],
                            start=st, stop=sp,
                        )
            for n in range(4):
                nc.vector.tensor_copy(k_sb[:, 512 * n:512 * (n + 1)], ps_k[n][:])
            for n in range(4):
                nc.vector.tensor_copy(vt_sb[:, 512 * n:512 * (n + 1)], ps_vt[n][:])

            # ---- phase 2: Q^T (even positions) + V tiles (DMA transpose) ----
            # banks 0-1: Q accum
            ps_q = [bank(n, name="psq") for n in range(2)]
            for j in range(NCT):
                g, jj = j // XJ, j % XJ
                st, sp = j == 0, j == NCT - 1
                xq_j = xg_sb[g][:, jj, :].rearrange("p (m two) -> p m two", two=2 * P)
                for n in range(2):
                    nc.tensor.matmul(
                        ps_q[n][:],
                        lhsT=wq_sb[:, j, :],
                        rhs=xq_j[:, 4 * n:4 * (n + 1), 0:P],
                        start=st, stop=sp,
                    )
            for n in range(2):
                nc.vector.tensor_copy(q_sb[:, 512 * n:512 * (n + 1)], ps_q[n][:])

            # V tiles [k, h] via XBAR DMA transpose (SBUF->SBUF, no PE time)
            for kt in range(NKT):
                nc.sync.dma_start_transpose(
                    v_sb[:, kt, :], vt_sb[:, kt * P:(kt + 1) * P]
                )

            # ---- phase 3: attention, kt-outer over two class halves ----
            # banks 4-6: S tiles rotate; bank 7: OT half A; bank 2: den half A
            # bank 0: OT half B; bank 1: den half B (after Q frees them)
            ps_ot = [bank(7, name="psotA"), bank(0, name="psotB")]
            # den reduced with a [128,128] ones lhsT: M=128 matmuls run ~2x
            # faster than M=1, all output rows carry the same column sums
            ps_den = [
                bank(2, name="psdenA"),
                bank(1, name="psdenB"),
            ]

            def attention_half(half):
                lo, hi = 512 * half, 512 * (half + 1)
                nkt = 8 * (half + 1)
                npairs = nkt // 2
                pairs = []  # (pair_sb, c0, n); den matmuls lag one pair

                def den_matmul(pr):
                    pair_sb, c0, n = pairs[pr]
                    nc.tensor.matmul(
                        ps_den[half][:, c0 - lo:512],
                        lhsT=ones_sb[:],
                        rhs=pair_sb[:, 0:n],
                        start=pr == 0, stop=pr == npairs - 1,
                    )

                # S-bank rotation: half1 reuses half0's finished OT (7) and
                # den (2) banks for a deeper exp/matmul pipeline on the short
                # trailing windows
                srot = [3, 4, 5, 6] if half == 0 else [3, 4, 5, 6, 7, 2]
                sbank = [0]

                def s_bank():
                    b = bank(srot[sbank[0] % len(srot)], name="pss")
                    sbank[0] += 1
                    return b

                def mask_add(ps_s, col, kt):
                    # additive -1e4 mask on that kt's first 128-col block,
                    # applied in PSUM so ACT->PE needs no vector hop
                    nc.tensor.matmul(
                        ps_s[:, col:col + P],
                        lhsT=ident[:],
                        rhs=negtri[:] if kt % 2 == 0 else oddneg[:],
                        start=False, stop=True,
                    )

                for pr in range(npairs):
                    c0 = max(P * pr, lo)
                    n = hi - c0
                    diag = c0 == P * pr  # diagonal class is in this half
                    kt0, kt1 = 2 * pr, 2 * pr + 1
                    pair_sb = pairp.tile([P, 512], dt.bfloat16, name="pair_sb")
                    if False:  # exp-merge packing: measured slower (pipeline)
                        # pack both S tiles into one bank: a single exp covers
                        # the pair (fewer ACT calls on the bottleneck engine)
                        ps_s = s_bank()
                        nc.tensor.matmul(
                            ps_s[:, 0:n],
                            lhsT=k_sb[:, kt0 * P:(kt0 + 1) * P],
                            rhs=q_sb[:, c0:hi],
                            start=True, stop=not diag,
                        )
                        if diag:
                            mask_add(ps_s, 0, kt0)
                        nc.tensor.matmul(
                            ps_s[:, n:2 * n],
                            lhsT=k_sb[:, kt1 * P:(kt1 + 1) * P],
                            rhs=q_sb[:, c0:hi],
                            start=True, stop=not diag,
                        )
                        if diag:
                            mask_add(ps_s, n, kt1)
                        a_sb = ephem.tile([P, 512], dt.bfloat16, name="a_sb")
                        nc.scalar.activation(
                            a_sb[:, 0:2 * n], ps_s[:, 0:2 * n],
                            mybir.ActivationFunctionType.Exp,
                            scale=SCALE,
                        )
                        nc.tensor.matmul(
                            ps_ot[half][:, c0 - lo:512],
                            lhsT=v_sb[:, kt0, :],
                            rhs=a_sb[:, 0:n],
                            start=kt0 == 0, stop=False,
                        )
                        nc.tensor.matmul(
                            ps_ot[half][:, c0 - lo:512],
                            lhsT=v_sb[:, kt1, :],
                            rhs=a_sb[:, n:2 * n],
                            start=False, stop=kt1 == nkt - 1,
                        )
                        nc.vector.tensor_add(
                            pair_sb[:, 0:n], a_sb[:, 0:n], a_sb[:, n:2 * n]
                        )
                    else:
                        a_pair = []
                        for kt in (kt0, kt1):
                            ps_s = s_bank()
                            nc.tensor.matmul(
                                ps_s[:, 0:n],
                                lhsT=k_sb[:, kt * P:(kt + 1) * P],
                                rhs=q_sb[:, c0:hi],
                                start=True, stop=not diag,
                            )
                            if diag:
                                mask_add(ps_s, 0, kt)
                            a_sb = ephem.tile([P, 512], dt.bfloat16, name="a_sb")
                            nc.scalar.activation(
                                a_sb[:, 0:n], ps_s[:, 0:n],
                                mybir.ActivationFunctionType.Exp,
                                scale=SCALE,
                            )
                            nc.tensor.matmul(
                                ps_ot[half][:, c0 - lo:512],
                                lhsT=v_sb[:, kt, :],
                                rhs=a_sb[:, 0:n],
                                start=kt == 0, stop=kt == nkt - 1,
                            )
                            a_pair.append(a_sb)
                        # den: bf16 pair-sum on VECTOR (halves the den passes)
                        nc.vector.tensor_add(
                            pair_sb[:, 0:n], a_pair[0][:, 0:n], a_pair[1][:, 0:n]
                        )
                    pairs.append((pair_sb, c0, n))
                    if pr > 0:
                        den_matmul(pr - 1)
                den_matmul(npairs - 1)

                ot_sb = outp.tile([P, 512], dt.bfloat16, name="ot_sb")
                nc.vector.tensor_copy(ot_sb[:], ps_ot[half][:])
                nc.sync.dma_start(out=ot_d[:, lo:hi], in_=ot_sb[:])
                # den copy on the scalar engine: runs parallel to the vector
                # ot cast, shortening the last-engine tail before teardown
                den_sb = outp.tile([1, 512], dt.float32, name="den_sb")
                nc.scalar.copy(den_sb[:], ps_den[half][0:1, :])
                nc.sync.dma_start(out=den_d[:, lo:hi], in_=den_sb[:])

            attention_half(0)
            attention_half(1)

    nc.compile()
    return nc


def _core_tiles(core):
    par = core % 2
    return [2 * (i - 1) + par for i in range(1, NQT + 1)]


def _prep_inputs(x, Wq, Wk, Wv):
    """Build the 8 per-core input maps."""
    def wshape(w):
        # [C, H] -> [128, NCT, H]: w_r[p, j, h] = w[j*128 + p, h]
        return np.ascontiguousarray(
            w.astype(BF16).reshape(NCT, P, H).transpose(1, 0, 2)
        )

    wq_b, wk_b, wv_b = wshape(Wq), wshape(Wk), wshape(Wv)
    x_bf = x.astype(BF16)

    in_maps = []
    for core in range(N_CORES):
        b, par = core // 2, core % 2
        # position -> global key tile: [own_1, sib_1, own_2, sib_2, ...]
        perm = []
        for m in range(NQT):
            perm += [2 * m + par, 2 * m + 1 - par]
        cols = np.concatenate([np.arange(P * t, P * t + P) for t in perm])
        xT = np.ascontiguousarray(x_bf[b].T[:, cols])
        oddn = np.full((P, 1), -10000.0 * (1 - par), np.float32)
        in_maps.append({
            "xkvT": xT,
            "wq": wq_b, "wk": wk_b, "wv": wv_b,
            "oddn": np.ascontiguousarray(oddn),
        })
    return in_maps


def _assemble(results):
    out = np.empty((B, T, H), np.float32)
    for core in range(N_CORES):
        r = results[core]
        o = (np.asarray(r["ot"], np.float32) / r["den"]).T  # [NQ, H]
        for idx, t in enumerate(_core_tiles(core)):
            out[core // 2, P * t:P * (t + 1), :] = o[P * idx:P * (idx + 1), :]
    return out


def _run(inputs, trace=False, **spmd_kwargs):
    from concourse.bass_utils import run_bass_kernel_spmd

    if "nc" not in _cache:
        _cache["nc"] = _build()
    nc = _cache["nc"]
    in_maps = _prep_inputs(
        np.asarray(inputs["x"], np.float32),
        np.asarray(inputs["Wq"], np.float32),
        np.asarray(inputs["Wk"], np.float32),
        np.asarray(inputs["Wv"], np.float32),
    )
    res = run_bass_kernel_spmd(
        nc, in_maps, list(range(N_CORES)), trace=trace, **spmd_kwargs
    )
    return _assemble(res.results), res


def kernel(x, Wq, Wk, Wv):
    out, _ = _run({"x": x, "Wq": Wq, "Wk": Wk, "Wv": Wv})
    return out


# revision 41
# speedup vs baseline: 1.0104x; 1.0104x over previous
"""Single-head causal attention (B=4, T=2048, C=2048, H=128) on 8 TRN2 cores.

Sharding: 2 cores per batch. T is split into 16 query tiles of 128 rows.
Core (2b + par) handles batch b and query tiles t in {par, par+2, ..., par+14}.
Query tile class i (i = 1..8) is processed with a padded causal key window of
2i key tiles, so every core executes an identical program; per-core inputs
carry the asymmetry.

Key-order permutation trick: the host reorders the T dimension of the per-core
x.T buffer as [own_1, sib_1, own_2, sib_2, ...] (own_i = the core's class-i
query tile, sib_i = the sibling core's). Attention sums are order-invariant
over keys, and the class-i key window is exactly the first 2i positions of
this order, so the program is position-based and identical across cores:
  - Q columns are the even positions (fixed offsets for every core),
  - the window's second-to-last position (even) is always the diagonal tile
    (constant triangular mask, built on device),
  - the last position (odd, the sibling tile) is all-allowed or all-masked
    depending only on core parity (a per-core [128,1] scalar input).

Per core on-device pipeline (all matmuls bf16, fp32 PSUM accumulation):
  phase 1 (DMA-paced): x.T arrives in 8 chunks, the first split in half so the
    first matmul waits on ~1MB. The weight stream (sync ring) and x stream
    (scalar ring) share the bandwidth-saturated DMA queues evenly, so each
    ring's order is arranged to deliver just in time (wq/odd trail the x
    stream). K^T, V^T accumulate per chunk.
  phase 2: Q^T (even positions, strided rhs AP); V tiles [k, h] via XBAR DMA
    transpose from the V^T staging (SBUF->SBUF, zero PE time)
  phase 3: attention, kt-outer over two class halves (cols [0,512), [512,1024)):
    S^T(kt) = K_kt.T @ Q^T[:, c0:]     (one N<=512 matmul)
    diagonal-class masks are ADDED (-1e4) into the S PSUM via an
    identity-lhsT matmul (negtri for even kt, the parity-scalar broadcast for
    odd kt), so exp -> O needs no vector hop
    A = exp(scale * S^T) on ACT (softmax max-shift skipped: |s| < ~6)
    O^T[half] += V_kt(as lhsT) @ A
    den: per kt PAIR, bf16 pair-sum of the two A tiles on VECTOR, then a
    [128,128]-ones matmul per pair (M=128 runs ~2x faster than an M=1 row
    reduce; row 0 of the result is the column sum), emitted one pair behind
    so a lagging vector add never stalls the in-order PE stream.
  outputs: ot [128, 1024] bf16 (host divides in fp32), den [1, 1024] f32 via
  the scalar engine (parallel with vector's ot cast); host computes
  (ot/den).T and scatters rows back.

PSUM is managed as one pool with 8 explicitly reused bank tags: phase 1
K 0-3 / V^T 4-7; Q 0-1; attention S rotates 3-6 (half1: 3-6,7,2), OT half0
on 7 / half1 on 0, den half0 on 2 / half1 on 1.

Cross-core notes: collective_compute and remote_dma both fault in this
sandbox's PJRT path (no communicator; NRT_EXEC_UNIT_UNRECOVERABLE), so the
K/V projection duplication across the core pair cannot be exchanged away.
"""

import numpy as np
import ml_dtypes

B, T, C, H = 4, 2048, 2048, 128
P = 128                 # tile edge
NCT = C // P            # 16 contraction chunks
NKT = T // P            # 16 key tiles / positions
NQT = 8                 # query tile classes per core
NQ = NQT * P            # 1024 query rows per core
N_CORES = 8
SCALE = float(H) ** -0.5
BF16 = ml_dtypes.bfloat16

_cache = {}


def _build():
    import concourse.bass as bass
    import concourse.mybir as mybir
    import concourse.tile as tile
    from concourse import bacc
    from concourse.masks import make_identity, make_lower_triangular

    dt = mybir.dt
    nc = bacc.Bacc(
        "TRN2",
        target_bir_lowering=False,
        debug=False,
        enable_asserts=False,
        num_devices=N_CORES,
    )

    xkvT = nc.dram_tensor("xkvT", [C, T], dt.bfloat16, kind="ExternalInput").ap()
    wq_d = nc.dram_tensor("wq", [P, NCT, H], dt.bfloat16, kind="ExternalInput").ap()
    wk_d = nc.dram_tensor("wk", [P, NCT, H], dt.bfloat16, kind="ExternalInput").ap()
    wv_d = nc.dram_tensor("wv", [P, NCT, H], dt.bfloat16, kind="ExternalInput").ap()
    # parity mask value: 0.0 if the sibling (odd-position) key tile is allowed
    # (par=1 cores), -10000.0 if masked (par=0 cores) -- added to scores
    # pre-exp via a PSUM-accumulating matmul
    oddn_d = nc.dram_tensor("oddn", [P, 1], dt.float32, kind="ExternalInput").ap()
    ot_d = nc.dram_tensor("ot", [H, NQ], dt.bfloat16, kind="ExternalOutput").ap()
    den_d = nc.dram_tensor("den", [1, NQ], dt.float32, kind="ExternalOutput").ap()

    XJ = 2          # c-tiles per x chunk
    NG = NCT // XJ  # 8 pipelined load/compute chunks

    with tile.TileContext(nc) as tc:
        with (
            tc.tile_pool(name="persist", bufs=1) as persist,
            tc.tile_pool(name="ephem", bufs=16) as ephem,
            tc.tile_pool(name="pair", bufs=8) as pairp,
            tc.tile_pool(name="outp", bufs=2) as outp,
            tc.tile_pool(name="psum", bufs=1, space="PSUM") as psum,
        ):
            def bank(b, shape=(P, 512), dtype=dt.float32, name="pb"):
                return psum.tile(list(shape), dtype, tag=f"bank{b}", name=f"{name}{b}")

            wq_sb = persist.tile([P, NCT, H], dt.bfloat16)
            wk_sb = persist.tile([P, NCT, H], dt.bfloat16)
            wv0_sb = persist.tile([P, 1, H], dt.bfloat16)
            wvr_sb = persist.tile([P, NCT - 1, H], dt.bfloat16)
            oddn_sb = persist.tile([P, 1], dt.float32)
            xg_sb = [
                persist.tile([P, XJ, T], dt.bfloat16, name=f"xg{g}")
                for g in range(NG)
            ]
            k_sb = persist.tile([P, T], dt.bfloat16)       # K^T [h, T]
            vt_sb = persist.tile([P, T], dt.bfloat16)      # V^T [h, T]
            v_sb = persist.tile([P, NKT, H], dt.bfloat16)  # V tiles [k, h]
            q_sb = persist.tile([P, NQ], dt.bfloat16)      # Q^T [h, NQ]
            ident = persist.tile([P, P], dt.bfloat16)
            negtri = persist.tile([P, P], dt.bfloat16)     # -1e4 where k > q
            oddneg = persist.tile([P, P], dt.bfloat16)     # oddn broadcast
            ones_sb = persist.tile([P, P], dt.bfloat16)    # den reducer lhsT

            def xdma(g):
                nc.scalar.dma_start(
                    out=xg_sb[g][:],
                    in_=xkvT[XJ * P * g:XJ * P * (g + 1), :].rearrange(
                        "(j p) t -> p j t", p=P
                    ),
                )

            def xdma_half(jj):
                nc.scalar.dma_start(
                    out=xg_sb[0][:, jj:jj + 1, :],
                    in_=xkvT[P * jj:P * (jj + 1), :]
                    .rearrange("(j p) t -> p j t", p=P),
                )

            # The 16 DMA queues are bandwidth-saturated through phase 1 and
            # the two HWDGE rings (sync: weights, scalar: x) share them about
            # evenly while both have work. Startup critical path: wk + the
            # first half of chunk 0 (~1MB) gates the first matmul; the wv
            # sliver covers chunk 0's V matmuls; wq/odd trail the x stream
            # (needed ~30us later, and by then the sync ring is drained so x
            # runs at full rate).
            nc.sync.dma_start(out=wk_sb[:], in_=wk_d[:])
            xdma_half(0)
            nc.sync.dma_start(out=wv0_sb[:], in_=wv_d[:, 0:1, :])
            xdma_half(1)
            nc.sync.dma_start(out=wvr_sb[:], in_=wv_d[:, 1:NCT, :])
            for g in range(1, NG):
                xdma(g)
            nc.scalar.dma_start(out=wq_sb[:], in_=wq_d[:])
            nc.scalar.dma_start(out=oddn_sb[:], in_=oddn_d[:])
            make_identity(nc, ident[:])
            make_lower_triangular(nc, negtri[:], val=-10000.0, diag=False)
            nc.vector.memset(oddneg[:], 1.0)
            nc.vector.tensor_scalar_mul(oddneg[:], oddneg[:], oddn_sb[:])
            nc.vector.memset(ones_sb[:], 1.0)
            # preload the ACT exp table off the attention critical path
            warm_sb = persist.tile([P, 1], dt.float32)
            nc.scalar.activation(
                warm_sb[:], ones_sb[:, 0:1], mybir.ActivationFunctionType.Exp
            )

            # ---- phase 1: pipelined x load + K^T / V^T accumulation ----
            # banks 0-3: K accum; banks 4-7: V^T accum
            ps_k = [bank(n, name="psk") for n in range(4)]
            ps_vt = [bank(4 + n, name="psvt") for n in range(4)]
            for g in range(NG):
                for jj in range(XJ):
                    j = XJ * g + jj
                    st, sp = j == 0, j == NCT - 1
                    wvj = wv0_sb[:, 0, :] if j == 0 else wvr_sb[:, j - 1, :]
                    for n in range(4):
                        nc.tensor.matmul(
                            ps_k[n][:],
                            lhsT=wk_sb[:, j, :],
                            rhs=xg_sb[g][:, jj, 512 * n:512 * (n + 1)],
                            start=st, stop=sp,
                        )
                    for n in range(4):
                        nc.tensor.matmul(
                            ps_vt[n][:],
                            lhsT=wvj,
                            rhs=xg_sb[g][:, jj, 512 * n:512 * (n + 1)],
                            start=st, stop=sp,
                        )
            for n in range(4):
                nc.vector.tensor_copy(k_sb[:, 512 * n:512 * (n + 1)], ps_k[n][:])
            for n in range(4):
                nc.vector.tensor_copy(vt_sb[:, 512 * n:512 * (n + 1)], ps_vt[n][:])

            # ---- phase 2: Q^T (even positions) + V tiles (DMA transpose) ----
            # banks 0-1: Q accum
            ps_q = [bank(n, name="psq") for n in range(2)]
            for j in range(NCT):
                g, jj = j // XJ, j % XJ
                st, sp = j == 0, j == NCT - 1
                xq_j = xg_sb[g][:, jj, :].rearrange("p (m two) -> p m two", two=2 * P)
                for n in range(2):
                    nc.tensor.matmul(
                        ps_q[n][:],
                        lhsT=wq_sb[:, j, :],
                        rhs=xq_j[:, 4 * n:4 * (n + 1), 0:P],
                        start=st, stop=sp,
                    )
            for n in range(2):
                nc.vector.tensor_copy(q_sb[:, 512 * n:512 * (n + 1)], ps_q[n][:])

            # V tiles [k, h] via XBAR DMA transpose (SBUF->SBUF, no PE time)
            for kt in range(NKT):
                nc.sync.dma_start_transpose(
                    v_sb[:, kt, :], vt_sb[:, kt * P:(kt + 1) * P]
                )

            # ---- phase 3: attention, kt-outer over two class halves ----
            # banks 4-6: S tiles rotate; bank 7: OT half A; bank 2: den half A
            # bank 0: OT half B; bank 1: den half B (after Q frees them)
            ps_ot = [bank(7, name="psotA"), bank(0, name="psotB")]
            # den reduced with a [128,128] ones lhsT: M=128 matmuls run ~2x
            # faster than M=1, all output rows carry the same column sums
            ps_den = [
                bank(2, name="psdenA"),
                bank(1, name="psdenB"),
            ]

            def attention_half(half):
                lo, hi = 512 * half, 512 * (half + 1)
                nkt = 8 * (half + 1)
                npairs = nkt // 2
                pairs = []  # (pair_sb, c0, n); den matmuls lag one pair

                def den_matmul(pr):
                    pair_sb, c0, n = pairs[pr]
                    nc.tensor.matmul(
                        ps_den[half][:, c0 - lo:512],
                        lhsT=ones_sb[:],
                        rhs=pair_sb[:, 0:n],
                        start=pr == 0, stop=pr == npairs - 1,
                    )

                # S-bank rotation: half1 reuses half0's finished OT (7) and
                # den (2) banks for a deeper exp/matmul pipeline on the short
                # trailing windows
                srot = [3, 4, 5, 6] if half == 0 else [3, 4, 5, 6, 7, 2]
                sbank = [0]

                def s_bank():
                    b = bank(srot[sbank[0] % len(srot)], name="pss")
                    sbank[0] += 1
                    return b

                def mask_add(ps_s, col, kt):
                    # additive -1e4 mask on that kt's first 128-col block,
                    # applied in PSUM so ACT->PE needs no vector hop
                    nc.tensor.matmul(
                        ps_s[:, col:col + P],
                        lhsT=ident[:],
                        rhs=negtri[:] if kt % 2 == 0 else oddneg[:],
                        start=False, stop=True,
                    )

                # hoist S/exp of the short trailing pairs to the head of the
                # half: the 6-deep S-bank queue hides their ACT latency there,
                # removing the exp-latency-chained PE stalls at the drain. The
                # O matmuls stay at their kt position (accumulation order).
                hoist = {6, 7} if half == 1 else set()
                pre_a = {}

                def s_exp(pr, c0, n, diag):
                    a_two = []
                    for kt in (2 * pr, 2 * pr + 1):
                        ps_s = s_bank()
                        nc.tensor.matmul(
                            ps_s[:, 0:n],
                            lhsT=k_sb[:, kt * P:(kt + 1) * P],
                            rhs=q_sb[:, c0:hi],
                            start=True, stop=not diag,
                        )
                        if diag:
                            mask_add(ps_s, 0, kt)
                        a_sb = ephem.tile([P, 512], dt.bfloat16, name="a_sb")
                        nc.scalar.activation(
                            a_sb[:, 0:n], ps_s[:, 0:n],
                            mybir.ActivationFunctionType.Exp,
                            scale=SCALE,
                        )
                        a_two.append(a_sb)
                    return a_two

                for pr in sorted(hoist):
                    c0 = max(P * pr, lo)
                    pre_a[pr] = s_exp(pr, c0, hi - c0, c0 == P * pr)

                for pr in range(npairs):
                    c0 = max(P * pr, lo)
                    n = hi - c0
                    diag = c0 == P * pr  # diagonal class is in this half
                    kt0, kt1 = 2 * pr, 2 * pr + 1
                    pair_sb = pairp.tile([P, 512], dt.bfloat16, name="pair_sb")
                    if pr in pre_a:
                        a_pair = pre_a[pr]
                        for kt, a_sb in zip((kt0, kt1), a_pair):
                            nc.tensor.matmul(
                                ps_ot[half][:, c0 - lo:512],
                                lhsT=v_sb[:, kt, :],
                                rhs=a_sb[:, 0:n],
                                start=kt == 0, stop=kt == nkt - 1,
                            )
                        nc.vector.tensor_add(
                            pair_sb[:, 0:n], a_pair[0][:, 0:n], a_pair[1][:, 0:n]
                        )
                        pairs.append((pair_sb, c0, n))
                        if pr > 0:
                            den_matmul(pr - 1)
                        continue
                    if False:  # exp-merge packing: measured slower (pipeline)
                        # pack both S tiles into one bank: a single exp covers
                        # the pair (fewer ACT calls on the bottleneck engine)
                        ps_s = s_bank()
                        nc.tensor.matmul(
                            ps_s[:, 0:n],
                            lhsT=k_sb[:, kt0 * P:(kt0 + 1) * P],
                            rhs=q_sb[:, c0:hi],
                            start=True, stop=not diag,
                        )
                        if diag:
                            mask_add(ps_s, 0, kt0)
                        nc.tensor.matmul(
                            ps_s[:, n:2 * n],
                            lhsT=k_sb[:, kt1 * P:(kt1 + 1) * P],
                            rhs=q_sb[:, c0:hi],
                            start=True, stop=not diag,
                        )
                        if diag:
                            mask_add(ps_s, n, kt1)
                        a_sb = ephem.tile([P, 512], dt.bfloat16, name="a_sb")
                        nc.scalar.activation(
                            a_sb[:, 0:2 * n], ps_s[:, 0:2 * n],
                            mybir.ActivationFunctionType.Exp,
                            scale=SCALE,
                        )
                        nc.tensor.matmul(
                            ps_ot[half][:, c0 - lo:512],
                            lhsT=v_sb[:, kt0, :],
                            rhs=a_sb[:, 0:n],
                            start=kt0 == 0, stop=False,
                        )
                        nc.tensor.matmul(
                            ps_ot[half][:, c0 - lo:512],
                            lhsT=v_sb[:, kt1, :],
                            rhs=a_sb[:, n:2 * n],
                            start=False, stop=kt1 == nkt - 1,
                        )
                        nc.vector.tensor_add(
                            pair_sb[:, 0:n], a_sb[:, 0:n], a_sb[:, n:2 * n]
                        )
                    else:
                        a_pair = []
                        for kt in (kt0, kt1):
                            ps_s = s_bank()
                            nc.tensor.matmul(
                                ps_s[:, 0:n],
                                lhsT=k_sb[:, kt * P:(kt + 1) * P],
                                rhs=q_sb[:, c0:hi],
                                start=True, stop=not diag,
                            )
                            if diag:
                                mask_add(ps_s, 0, kt)
                            a_sb = ephem.tile([P, 512], dt.bfloat16, name="a_sb")
                            nc.scalar.activation(
                                a_sb[:, 0:n], ps_s[:, 0:n],
                                mybir.ActivationFunctionType.Exp,
                                scale=SCALE,
                            )
                            nc.tensor.matmul(
                                ps_ot[half][:, c0 - lo:512],
                                lhsT=v_sb[:, kt, :],
                                rhs=a_sb[:, 0:n],
                                start=kt == 0, stop=kt == nkt - 1,
                            )
                            a_pair.append(a_sb)
                        # den: bf16 pair-sum on VECTOR (halves the den passes)
                        nc.vector.tensor_add(
                            pair_sb[:, 0:n], a_pair[0][:, 0:n], a_pair[1][:, 0:n]
                        )
                    pairs.append((pair_sb, c0, n))
                    if pr > 0:
                        den_matmul(pr - 1)
                den_matmul(npairs - 1)

                ot_sb = outp.tile([P, 512], dt.bfloat16, name="ot_sb")
                nc.vector.tensor_copy(ot_sb[:], ps_ot[half][:])
                nc.sync.dma_start(out=ot_d[:, lo:hi], in_=ot_sb[:])
                # den copy on the scalar engine: runs parallel to the vector
                # ot cast, shortening the last-engine tail before teardown
                den_sb = outp.tile([1, 512], dt.float32, name="den_sb")
                nc.scalar.copy(den_sb[:], ps_den[half][0:1, :])
                nc.sync.dma_start(out=den_d[:, lo:hi], in_=den_sb[:])

            attention_half(0)
            attention_half(1)

    nc.compile()
    return nc


def _core_tiles(core):
    par = core % 2
    return [2 * (i - 1) + par for i in range(1, NQT + 1)]


def _prep_inputs(x, Wq, Wk, Wv):
    """Build the 8 per-core input maps."""
    def wshape(w):
        # [C, H] -> [128, NCT, H]: w_r[p, j, h] = w[j*128 + p, h]
        return np.ascontiguousarray(
            w.astype(BF16).reshape(NCT, P, H).transpose(1, 0, 2)
        )

    wq_b, wk_b, wv_b = wshape(Wq), wshape(Wk), wshape(Wv)
    x_bf = x.astype(BF16)

    in_maps = []
    for core in range(N_CORES):
        b, par = core // 2, core % 2
        # position -> global key tile: [own_1, sib_1, own_2, sib_2, ...]
        perm = []
        for m in range(NQT):
            perm += [2 * m + par, 2 * m + 1 - par]
        cols = np.concatenate([np.arange(P * t, P * t + P) for t in perm])
        xT = np.ascontiguousarray(x_bf[b].T[:, cols])
        oddn = np.full((P, 1), -10000.0 * (1 - par), np.float32)
        in_maps.append({
            "xkvT": xT,
            "wq": wq_b, "wk": wk_b, "wv": wv_b,
            "oddn": np.ascontiguousarray(oddn),
        })
    return in_maps


def _assemble(results):
    out = np.empty((B, T, H), np.float32)
    for core in range(N_CORES):
        r = results[core]
        o = (np.asarray(r["ot"], np.float32) / r["den"]).T  # [NQ, H]
        for idx, t in enumerate(_core_tiles(core)):
            out[core // 2, P * t:P * (t + 1), :] = o[P * idx:P * (idx + 1), :]
    return out


def _run(inputs, trace=False, **spmd_kwargs):
    from concourse.bass_utils import run_bass_kernel_spmd

    if "nc" not in _cache:
        _cache["nc"] = _build()
    nc = _cache["nc"]
    in_maps = _prep_inputs(
        np.asarray(inputs["x"], np.float32),
        np.asarray(inputs["Wq"], np.float32),
        np.asarray(inputs["Wk"], np.float32),
        np.asarray(inputs["Wv"], np.float32),
    )
    res = run_bass_kernel_spmd(
        nc, in_maps, list(range(N_CORES)), trace=trace, **spmd_kwargs
    )
    return _assemble(res.results), res


def kernel(x, Wq, Wk, Wv):
    out, _ = _run({"x": x, "Wq": Wq, "Wk": Wk, "Wv": Wv})
    return out


# revision 42
# speedup vs baseline: 1.0176x; 1.0071x over previous
"""Single-head causal attention (B=4, T=2048, C=2048, H=128) on 8 TRN2 cores.

Sharding: 2 cores per batch. T is split into 16 query tiles of 128 rows.
Core (2b + par) handles batch b and query tiles t in {par, par+2, ..., par+14}.
Query tile class i (i = 1..8) is processed with a padded causal key window of
2i key tiles, so every core executes an identical program; per-core inputs
carry the asymmetry.

Key-order permutation trick: the host reorders the T dimension of the per-core
x.T buffer as [own_1, sib_1, own_2, sib_2, ...] (own_i = the core's class-i
query tile, sib_i = the sibling core's). Attention sums are order-invariant
over keys, and the class-i key window is exactly the first 2i positions of
this order, so the program is position-based and identical across cores:
  - Q columns are the even positions (fixed offsets for every core),
  - the window's second-to-last position (even) is always the diagonal tile
    (constant triangular mask, built on device),
  - the last position (odd, the sibling tile) is all-allowed or all-masked
    depending only on core parity (a per-core [128,1] scalar input).

Per core on-device pipeline (all matmuls bf16, fp32 PSUM accumulation):
  phase 1 (DMA-paced): x.T arrives in 8 chunks, the first split in half so the
    first matmul waits on ~1MB. The weight stream (sync ring) and x stream
    (scalar ring) share the bandwidth-saturated DMA queues evenly, so each
    ring's order is arranged to deliver just in time (wq/odd trail the x
    stream). K^T, V^T accumulate per chunk.
  phase 2: Q^T (even positions, strided rhs AP); V tiles [k, h] via XBAR DMA
    transpose from the V^T staging (SBUF->SBUF, zero PE time)
  phase 3: attention, kt-outer over two class halves (cols [0,512), [512,1024)):
    S^T(kt) = K_kt.T @ Q^T[:, c0:]     (one N<=512 matmul)
    diagonal-class masks are ADDED (-1e4) into the S PSUM via an
    identity-lhsT matmul (negtri for even kt, the parity-scalar broadcast for
    odd kt), so exp -> O needs no vector hop
    A = exp(scale * S^T) on ACT (softmax max-shift skipped: |s| < ~6)
    O^T[half] += V_kt(as lhsT) @ A
    den: per kt PAIR, bf16 pair-sum of the two A tiles on VECTOR, then a
    [128,128]-ones matmul per pair (M=128 runs ~2x faster than an M=1 row
    reduce; row 0 of the result is the column sum), emitted one pair behind
    so a lagging vector add never stalls the in-order PE stream.
  outputs: ot [128, 1024] bf16 (host divides in fp32), den [1, 1024] f32 via
  the scalar engine (parallel with vector's ot cast); host computes
  (ot/den).T and scatters rows back.

PSUM is managed as one pool with 8 explicitly reused bank tags: phase 1
K 0-3 / V^T 4-7; Q 0-1; attention S rotates 3-6 (half1: 3-6,7,2), OT half0
on 7 / half1 on 0, den half0 on 2 / half1 on 1.

Cross-core notes: collective_compute and remote_dma both fault in this
sandbox's PJRT path (no communicator; NRT_EXEC_UNIT_UNRECOVERABLE), so the
K/V projection duplication across the core pair cannot be exchanged away.
"""

import numpy as np
import ml_dtypes

B, T, C, H = 4, 2048, 2048, 128
P = 128                 # tile edge
NCT = C // P            # 16 contraction chunks
NKT = T // P            # 16 key tiles / positions
NQT = 8                 # query tile classes per core
NQ = NQT * P            # 1024 query rows per core
N_CORES = 8
SCALE = float(H) ** -0.5
BF16 = ml_dtypes.bfloat16

_cache = {}


def _build():
    import concourse.bass as bass
    import concourse.mybir as mybir
    import concourse.tile as tile
    from concourse import bacc
    from concourse.masks import make_identity, make_lower_triangular

    dt = mybir.dt
    nc = bacc.Bacc(
        "TRN2",
        target_bir_lowering=False,
        debug=False,
        enable_asserts=False,
        num_devices=N_CORES,
    )

    xkvT = nc.dram_tensor("xkvT", [C, T], dt.bfloat16, kind="ExternalInput").ap()
    wq_d = nc.dram_tensor("wq", [P, NCT, H], dt.bfloat16, kind="ExternalInput").ap()
    wk_d = nc.dram_tensor("wk", [P, NCT, H], dt.bfloat16, kind="ExternalInput").ap()
    wv_d = nc.dram_tensor("wv", [P, NCT, H], dt.bfloat16, kind="ExternalInput").ap()
    # parity mask value: 0.0 if the sibling (odd-position) key tile is allowed
    # (par=1 cores), -10000.0 if masked (par=0 cores) -- added to scores
    # pre-exp via a PSUM-accumulating matmul
    oddn_d = nc.dram_tensor("oddn", [P, 1], dt.float32, kind="ExternalInput").ap()
    ot_d = nc.dram_tensor("ot", [H, NQ], dt.bfloat16, kind="ExternalOutput").ap()
    den_d = nc.dram_tensor("den", [1, NQ], dt.float32, kind="ExternalOutput").ap()

    XJ = 2          # c-tiles per x chunk
    NG = NCT // XJ  # 8 pipelined load/compute chunks

    with tile.TileContext(nc) as tc:
        with (
            tc.tile_pool(name="persist", bufs=1) as persist,
            tc.tile_pool(name="ephem", bufs=8) as ephem,
            tc.tile_pool(name="pair", bufs=8) as pairp,
            tc.tile_pool(name="outp", bufs=2) as outp,
            tc.tile_pool(name="psum", bufs=1, space="PSUM") as psum,
        ):
            def bank(b, shape=(P, 512), dtype=dt.float32, name="pb"):
                return psum.tile(list(shape), dtype, tag=f"bank{b}", name=f"{name}{b}")

            wq_sb = persist.tile([P, NCT, H], dt.bfloat16)
            wk_sb = persist.tile([P, NCT, H], dt.bfloat16)
            wv0_sb = persist.tile([P, 1, H], dt.bfloat16)
            wvr_sb = persist.tile([P, NCT - 1, H], dt.bfloat16)
            oddn_sb = persist.tile([P, 1], dt.float32)
            xg_sb = [
                persist.tile([P, XJ, T], dt.bfloat16, name=f"xg{g}")
                for g in range(NG)
            ]
            k_sb = persist.tile([P, T], dt.bfloat16)       # K^T [h, T]
            vt_sb = persist.tile([P, T], dt.bfloat16)      # V^T [h, T]
            v_sb = persist.tile([P, NKT, H], dt.bfloat16)  # V tiles [k, h]
            q_sb = persist.tile([P, NQ], dt.bfloat16)      # Q^T [h, NQ]
            ident = persist.tile([P, P], dt.bfloat16)
            negtri = persist.tile([P, P], dt.bfloat16)     # -1e4 where k > q
            oddneg = persist.tile([P, P], dt.bfloat16)     # oddn broadcast
            ones_sb = persist.tile([P, P], dt.bfloat16)    # den reducer lhsT

            def xdma(g):
                nc.scalar.dma_start(
                    out=xg_sb[g][:],
                    in_=xkvT[XJ * P * g:XJ * P * (g + 1), :].rearrange(
                        "(j p) t -> p j t", p=P
                    ),
                )

            def xdma_half(jj):
                nc.scalar.dma_start(
                    out=xg_sb[0][:, jj:jj + 1, :],
                    in_=xkvT[P * jj:P * (jj + 1), :]
                    .rearrange("(j p) t -> p j t", p=P),
                )

            # The 16 DMA queues are bandwidth-saturated through phase 1 and
            # the two HWDGE rings (sync: weights, scalar: x) share them about
            # evenly while both have work. Startup critical path: wk + the
            # first half of chunk 0 (~1MB) gates the first matmul; the wv
            # sliver covers chunk 0's V matmuls; wq/odd trail the x stream
            # (needed ~30us later, and by then the sync ring is drained so x
            # runs at full rate).
            nc.sync.dma_start(out=wk_sb[:], in_=wk_d[:])
            xdma_half(0)
            nc.sync.dma_start(out=wv0_sb[:], in_=wv_d[:, 0:1, :])
            xdma_half(1)
            nc.sync.dma_start(out=wvr_sb[:], in_=wv_d[:, 1:NCT, :])
            for g in range(1, NG):
                xdma(g)
            nc.scalar.dma_start(out=wq_sb[:], in_=wq_d[:])
            nc.scalar.dma_start(out=oddn_sb[:], in_=oddn_d[:])
            make_identity(nc, ident[:])
            make_lower_triangular(nc, negtri[:], val=-10000.0, diag=False)
            nc.vector.memset(oddneg[:], 1.0)
            nc.vector.tensor_scalar_mul(oddneg[:], oddneg[:], oddn_sb[:])
            nc.vector.memset(ones_sb[:], 1.0)
            # preload the ACT exp table off the attention critical path
            warm_sb = persist.tile([P, 1], dt.float32)
            nc.scalar.activation(
                warm_sb[:], ones_sb[:, 0:1], mybir.ActivationFunctionType.Exp
            )

            # ---- phase 1: pipelined x load + K^T / V^T accumulation ----
            # banks 0-3: K accum; banks 4-7: V^T accum
            ps_k = [bank(n, name="psk") for n in range(4)]
            ps_vt = [bank(4 + n, name="psvt") for n in range(4)]
            for g in range(NG):
                for jj in range(XJ):
                    j = XJ * g + jj
                    st, sp = j == 0, j == NCT - 1
                    wvj = wv0_sb[:, 0, :] if j == 0 else wvr_sb[:, j - 1, :]
                    for n in range(4):
                        nc.tensor.matmul(
                            ps_k[n][:],
                            lhsT=wk_sb[:, j, :],
                            rhs=xg_sb[g][:, jj, 512 * n:512 * (n + 1)],
                            start=st, stop=sp,
                        )
                    for n in range(4):
                        nc.tensor.matmul(
                            ps_vt[n][:],
                            lhsT=wvj,
                            rhs=xg_sb[g][:, jj, 512 * n:512 * (n + 1)],
                            start=st, stop=sp,
                        )
            for n in range(4):
                nc.vector.tensor_copy(k_sb[:, 512 * n:512 * (n + 1)], ps_k[n][:])
            for n in range(4):
                nc.vector.tensor_copy(vt_sb[:, 512 * n:512 * (n + 1)], ps_vt[n][:])

            # ---- phase 2: Q^T (even positions) + V tiles (DMA transpose) ----
            # banks 0-1: Q accum
            ps_q = [bank(n, name="psq") for n in range(2)]
            for j in range(NCT):
                g, jj = j // XJ, j % XJ
                st, sp = j == 0, j == NCT - 1
                xq_j = xg_sb[g][:, jj, :].rearrange("p (m two) -> p m two", two=2 * P)
                for n in range(2):
                    nc.tensor.matmul(
                        ps_q[n][:],
                        lhsT=wq_sb[:, j, :],
                        rhs=xq_j[:, 4 * n:4 * (n + 1), 0:P],
                        start=st, stop=sp,
                    )
            for n in range(2):
                nc.vector.tensor_copy(q_sb[:, 512 * n:512 * (n + 1)], ps_q[n][:])

            # V tiles [k, h] via XBAR DMA transpose (SBUF->SBUF, no PE time)
            for kt in range(NKT):
                nc.sync.dma_start_transpose(
                    v_sb[:, kt, :], vt_sb[:, kt * P:(kt + 1) * P]
                )

            # ---- phase 3: attention, kt-outer over two class halves ----
            # banks 4-6: S tiles rotate; bank 7: OT half A; bank 2: den half A
            # bank 0: OT half B; bank 1: den half B (after Q frees them)
            ps_ot = [bank(7, name="psotA"), bank(0, name="psotB")]
            # den reduced with a [128,128] ones lhsT: M=128 matmuls run ~2x
            # faster than M=1, all output rows carry the same column sums
            ps_den = [
                bank(2, name="psdenA"),
                bank(1, name="psdenB"),
            ]

            def attention_half(half):
                lo, hi = 512 * half, 512 * (half + 1)
                nkt = 8 * (half + 1)
                npairs = nkt // 2
                pairs = []  # (pair_sb, c0, n); den matmuls lag one pair

                def den_matmul(pr):
                    pair_sb, c0, n = pairs[pr]
                    nc.tensor.matmul(
                        ps_den[half][:, c0 - lo:512],
                        lhsT=ones_sb[:],
                        rhs=pair_sb[:, 0:n],
                        start=pr == 0, stop=pr == npairs - 1,
                    )

                # S-bank rotation: half1 reuses half0's finished OT (7) and
                # den (2) banks for a deeper exp/matmul pipeline on the short
                # trailing windows
                srot = [3, 4, 5, 6] if half == 0 else [3, 4, 5, 6, 7, 2]
                sbank = [0]

                def s_bank():
                    b = bank(srot[sbank[0] % len(srot)], name="pss")
                    sbank[0] += 1
                    return b

                def mask_add(ps_s, col, kt):
                    # additive -1e4 mask on that kt's first 128-col block,
                    # applied in PSUM so ACT->PE needs no vector hop
                    nc.tensor.matmul(
                        ps_s[:, col:col + P],
                        lhsT=ident[:],
                        rhs=negtri[:] if kt % 2 == 0 else oddneg[:],
                        start=False, stop=True,
                    )

                for pr in range(npairs):
                    c0 = max(P * pr, lo)
                    n = hi - c0
                    diag = c0 == P * pr  # diagonal class is in this half
                    kt0, kt1 = 2 * pr, 2 * pr + 1
                    pair_sb = pairp.tile([P, 512], dt.bfloat16, name="pair_sb")
                    if False:  # exp-merge packing: measured slower (pipeline)
                        # pack both S tiles into one bank: a single exp covers
                        # the pair (fewer ACT calls on the bottleneck engine)
                        ps_s = s_bank()
                        nc.tensor.matmul(
                            ps_s[:, 0:n],
                            lhsT=k_sb[:, kt0 * P:(kt0 + 1) * P],
                            rhs=q_sb[:, c0:hi],
                            start=True, stop=not diag,
                        )
                        if diag:
                            mask_add(ps_s, 0, kt0)
                        nc.tensor.matmul(
                            ps_s[:, n:2 * n],
                            lhsT=k_sb[:, kt1 * P:(kt1 + 1) * P],
                            rhs=q_sb[:, c0:hi],
                            start=True, stop=not diag,
                        )
                        if diag:
                            mask_add(ps_s, n, kt1)
                        a_sb = ephem.tile([P, 512], dt.bfloat16, name="a_sb")
                        nc.scalar.activation(
                            a_sb[:, 0:2 * n], ps_s[:, 0:2 * n],
                            mybir.ActivationFunctionType.Exp,
                            scale=SCALE,
                        )
                        nc.tensor.matmul(
                            ps_ot[half][:, c0 - lo:512],
                            lhsT=v_sb[:, kt0, :],
                            rhs=a_sb[:, 0:n],
                            start=kt0 == 0, stop=False,
                        )
                        nc.tensor.matmul(
                            ps_ot[half][:, c0 - lo:512],
                            lhsT=v_sb[:, kt1, :],
                            rhs=a_sb[:, n:2 * n],
                            start=False, stop=kt1 == nkt - 1,
                        )
                        nc.vector.tensor_add(
                            pair_sb[:, 0:n], a_sb[:, 0:n], a_sb[:, n:2 * n]
                        )
                    else:
                        a_pair = []
                        for kt in (kt0, kt1):
                            ps_s = s_bank()
                            nc.tensor.matmul(
                                ps_s[:, 0:n],
                                lhsT=k_sb[:, kt * P:(kt + 1) * P],
                                rhs=q_sb[:, c0:hi],
                                start=True, stop=not diag,
                            )
                            if diag:
                                mask_add(ps_s, 0, kt)
                            a_sb = ephem.tile([P, 512], dt.bfloat16, name="a_sb")
                            nc.scalar.activation(
                                a_sb[:, 0:n], ps_s[:, 0:n],
                                mybir.ActivationFunctionType.Exp,
                                scale=SCALE,
                            )
                            nc.tensor.matmul(
                                ps_ot[half][:, c0 - lo:512],
                                lhsT=v_sb[:, kt, :],
                                rhs=a_sb[:, 0:n],
                                start=kt == 0, stop=kt == nkt - 1,
                            )
                            a_pair.append(a_sb)
                        # den: bf16 pair-sum on VECTOR (halves the den passes)
                        nc.vector.tensor_add(
                            pair_sb[:, 0:n], a_pair[0][:, 0:n], a_pair[1][:, 0:n]
                        )
                    pairs.append((pair_sb, c0, n))
                    if pr > 0:
                        den_matmul(pr - 1)
                den_matmul(npairs - 1)

                ot_sb = outp.tile([P, 512], dt.bfloat16, name="ot_sb")
                nc.vector.tensor_copy(ot_sb[:], ps_ot[half][:])
                nc.sync.dma_start(out=ot_d[:, lo:hi], in_=ot_sb[:])
                # den copy on the scalar engine: runs parallel to the vector
                # ot cast, shortening the last-engine tail before teardown
                den_sb = outp.tile([1, 512], dt.float32, name="den_sb")
                nc.scalar.copy(den_sb[:], ps_den[half][0:1, :])
                nc.sync.dma_start(out=den_d[:, lo:hi], in_=den_sb[:])

            attention_half(0)
            attention_half(1)

    nc.compile()
    return nc


def _core_tiles(core):
    par = core % 2
    return [2 * (i - 1) + par for i in range(1, NQT + 1)]


def _prep_inputs(x, Wq, Wk, Wv):
    """Build the 8 per-core input maps."""
    def wshape(w):
        # [C, H] -> [128, NCT, H]: w_r[p, j, h] = w[j*128 + p, h]
        return np.ascontiguousarray(
            w.astype(BF16).reshape(NCT, P, H).transpose(1, 0, 2)
        )

    wq_b, wk_b, wv_b = wshape(Wq), wshape(Wk), wshape(Wv)
    x_bf = x.astype(BF16)

    in_maps = []
    for core in range(N_CORES):
        b, par = core // 2, core % 2
        # position -> global key tile: [own_1, sib_1, own_2, sib_2, ...]
        perm = []
        for m in range(NQT):
            perm += [2 * m + par, 2 * m + 1 - par]
        cols = np.concatenate([np.arange(P * t, P * t + P) for t in perm])
        xT = np.ascontiguousarray(x_bf[b].T[:, cols])
        oddn = np.full((P, 1), -10000.0 * (1 - par), np.float32)
        in_maps.append({
            "xkvT": xT,
            "wq": wq_b, "wk": wk_b, "wv": wv_b,
            "oddn": np.ascontiguousarray(oddn),
        })
    return in_maps


def _assemble(results):
    out = np.empty((B, T, H), np.float32)
    for core in range(N_CORES):
        r = results[core]
        o = (np.asarray(r["ot"], np.float32) / r["den"]).T  # [NQ, H]
        for idx, t in enumerate(_core_tiles(core)):
            out[core // 2, P * t:P * (t + 1), :] = o[P * idx:P * (idx + 1), :]
    return out


def _run(inputs, trace=False, **spmd_kwargs):
    from concourse.bass_utils import run_bass_kernel_spmd

    if "nc" not in _cache:
        _cache["nc"] = _build()
    nc = _cache["nc"]
    in_maps = _prep_inputs(
        np.asarray(inputs["x"], np.float32),
        np.asarray(inputs["Wq"], np.float32),
        np.asarray(inputs["Wk"], np.float32),
        np.asarray(inputs["Wv"], np.float32),
    )
    res = run_bass_kernel_spmd(
        nc, in_maps, list(range(N_CORES)), trace=trace, **spmd_kwargs
    )
    return _assemble(res.results), res


def kernel(x, Wq, Wk, Wv):
    out, _ = _run({"x": x, "Wq": Wq, "Wk": Wk, "Wv": Wv})
    return out
